# revision 1
# baseline (speedup 1.0000x reference)
"""GCN message-passing kernel (nn_Encoder_21646635172361) for 8 Trainium2 cores.

Math (reference):
    h   = x @ W.T                     [N,H]
    A~  = adjacency + self loops, symmetric-normalized: norm(r,c) = dinv[r]*dinv[c]
    out = PReLU(A~ @ h + b, alpha)

Key algebraic restructure: aggregation commutes with the linear transform,
    A~ @ (x W.T) = (A~ @ x) W.T
so we aggregate F=128-wide rows (4x less gather traffic than H=512), and
    agg[c] = dinv[c] * sum_{r->c} dinv[r] * x[r]
folds into: per-edge scaled one-hot selector matrices (carrying dinv[src])
contracted on the TensorEngine (scatter-add as matmul), and a per-dest scale
in the epilogue.

Distribution: destination nodes sharded round-robin (dst % 8) across the 8
cores; x/W/b/alpha replicated; each core gathers the source rows for its own
edges (DistGNN-style edge partition, no collectives needed).
"""

import os
import time
from contextlib import ExitStack

import numpy as np

N, F, H = 50000, 128, 512
NC_CORES = 8
ND = N // NC_CORES            # 6250 local dst nodes per core
WIN = 256                     # dst window width (matmul free dim; >=256 for f32r full rate)
NW = (ND + WIN - 1) // WIN    # 25 windows
NDP = NW * WIN                # 6400 padded local dst rows
SPLIT = 32768                 # int16 gather index split point
CH = 1024                     # gather chunk size (edges per dma_gather; >1024 fails on HW)
TILE = 128                    # edges per PE tile

# Results of the last kernel() call (for test.py introspection)
last_run_info = {}


def _plan(edge_index):
    """Host-side graph partition. Returns per-core device arrays + shared
    tile structure (uniform across cores, required for the SPMD program)."""
    src = np.asarray(edge_index[0], dtype=np.int64)
    dst = np.asarray(edge_index[1], dtype=np.int64)
    loops = np.arange(N, dtype=np.int64)
    src_all = np.concatenate([src, loops])
    dst_all = np.concatenate([dst, loops])

    deg = np.bincount(dst_all, minlength=N)
    dinv = (1.0 / np.sqrt(np.maximum(deg, 1).astype(np.float64))).astype(np.float32)
    dinv = np.where(deg > 0, dinv, 0.0).astype(np.float32)  # deg>=1 always (self loops)

    core = (dst_all % NC_CORES).astype(np.int64)
    loc = dst_all // NC_CORES          # local dst row
    win = loc // WIN
    dloc = (loc % WIN).astype(np.float32)
    low = src_all < SPLIT

    # group edges per (core, window, stream)
    grp = {}
    for k in range(NC_CORES):
        mk = core == k
        s_k, w_k, dl_k, lo_k = src_all[mk], win[mk], dloc[mk], low[mk]
        for w in range(NW):
            mw = w_k == w
            s_w, dl_w, lo_w = s_k[mw], dl_k[mw], lo_k[mw]
            grp[(k, w, 0)] = (s_w[lo_w], dl_w[lo_w])
            grp[(k, w, 1)] = (s_w[~lo_w] - SPLIT, dl_w[~lo_w])

    # uniform tile counts across cores
    T = np.zeros((2, NW), dtype=np.int64)
    for st in range(2):
        for w in range(NW):
            cnt = max(len(grp[(k, w, st)][0]) for k in range(NC_CORES))
            T[st, w] = (cnt + TILE - 1) // TILE
    tile_start = np.zeros((2, NW), dtype=np.int64)
    tile_start[0, 1:] = np.cumsum(T[0])[:-1]
    tile_start[1, 1:] = np.cumsum(T[1])[:-1]
    NT = [int(T[0].sum()), int(T[1].sum())]

    per_core = []
    for k in range(NC_CORES):
        core_dat = {}
        for st in range(2):
            nt = NT[st]
            idx = np.zeros(nt * TILE, dtype=np.int16)
            dsc = np.zeros(nt * TILE, dtype=np.float32)   # dinv[src], 0 for pads
            dlc = np.zeros(nt * TILE, dtype=np.float32)   # dest-local in window
            for w in range(NW):
                s_w, dl_w = grp[(k, w, st)]
                o = tile_start[st, w] * TILE
                n = len(s_w)
                idx[o:o + n] = s_w.astype(np.int16)
                gsrc = s_w + (SPLIT if st else 0)
                dsc[o:o + n] = dinv[gsrc]
                dlc[o:o + n] = dl_w
            # wrapped int16 index layout: [p, j] = idx[j*16 + p%16], replicated to 128 partitions
            wrapped = idx.reshape(-1, 16).T            # [16, nt*8]
            wrapped = np.tile(wrapped, (8, 1)).copy()  # [128, nt*8]
            core_dat[("idx", st)] = wrapped
            core_dat[("dsc", st)] = dsc.reshape(nt, TILE).T.copy()  # [128, nt]
            core_dat[("dlc", st)] = dlc.reshape(nt, TILE).T.copy()  # [128, nt]
        # dinv of this core's dst rows, [128, 2*NW] layout: [p, hw] = dinv(8*(128*hw+p)+k)
        locs = np.arange(NDP, dtype=np.int64)
        g = locs * NC_CORES + k
        dv = np.where(locs < ND, dinv[np.minimum(g, N - 1)], 0.0).astype(np.float32)
        core_dat["dinv_dst"] = dv.reshape(2 * NW, TILE).T.copy()    # [128, 50]
        per_core.append(core_dat)

    return per_core, T, tile_start, NT, dinv


def _build_program(T, tile_start, NT, fast_path):
    import concourse.bass as bass
    import concourse.mybir as mybir
    import concourse.tile as tile
    from concourse import bacc

    f32 = mybir.dt.float32
    f32r = mybir.dt.float32r
    i16 = mybir.dt.int16
    Alu = mybir.AluOpType
    Act = mybir.ActivationFunctionType

    nc = bacc.Bacc("TRN2", target_bir_lowering=False, debug=False,
                   num_devices=NC_CORES)

    x_d = nc.dram_tensor("x", [N, F], f32r, kind="ExternalInput").ap()
    wt_d = nc.dram_tensor("w_t", [F, H], f32r, kind="ExternalInput").ap()
    w1t_d = nc.dram_tensor("w1_t", [F, H], f32r, kind="ExternalInput").ap()
    idx_d = [nc.dram_tensor(f"idx{st}", [128, NT[st] * 8], i16,
                            kind="ExternalInput").ap() for st in range(2)]
    dsc_d = [nc.dram_tensor(f"dsc{st}", [128, NT[st]], f32,
                            kind="ExternalInput").ap() for st in range(2)]
    dlc_d = [nc.dram_tensor(f"dlc{st}", [128, NT[st]], f32,
                            kind="ExternalInput").ap() for st in range(2)]
    dvd_d = nc.dram_tensor("dinv_dst", [128, 2 * NW], f32, kind="ExternalInput").ap()
    if not fast_path:
        arow_d = nc.dram_tensor("alpha_row", [1, H], f32, kind="ExternalInput").ap()
        brow_d = nc.dram_tensor("b_row", [1, H], f32, kind="ExternalInput").ap()
    out_d = nc.dram_tensor("out", [NDP, H], f32, kind="ExternalOutput").ap()

    x_lo = x_d[0:SPLIT, :]
    x_hi = x_d[SPLIT:N, :]
    x_in = [x_lo, x_hi]

    n_chunks = [(NT[st] * TILE + CH - 1) // CH for st in range(2)]

    with tile.TileContext(nc) as tc, ExitStack() as ctx:
        cpool = ctx.enter_context(tc.tile_pool(name="const", bufs=1))
        gxpool = ctx.enter_context(tc.tile_pool(name="gx", bufs=4))
        ohpool = ctx.enter_context(tc.tile_pool(name="oh", bufs=8))
        aggpool = ctx.enter_context(tc.tile_pool(name="aggs", bufs=3))
        eppool = ctx.enter_context(tc.tile_pool(name="ep", bufs=3))
        ps_agg = ctx.enter_context(tc.tile_pool(name="ps_agg", bufs=2, space="PSUM"))
        ps_out = ctx.enter_context(tc.tile_pool(name="ps_out", bufs=2, space="PSUM"))

        # ---- one-time loads ----
        wt_sb = cpool.tile([F, H], f32r)
        nc.sync.dma_start(wt_sb[:], wt_d)
        if fast_path:
            w1t_sb = cpool.tile([F, H], f32r)
            nc.sync.dma_start(w1t_sb[:], w1t_d)
        idx_sb, dsc_sb, dlc_sb = [], [], []
        for st in range(2):
            t = cpool.tile([128, NT[st] * 8], i16, tag=f"idx{st}")
            nc.sync.dma_start(t[:], idx_d[st])
            idx_sb.append(t)
            t = cpool.tile([128, NT[st]], f32, tag=f"dsc{st}")
            nc.sync.dma_start(t[:], dsc_d[st])
            dsc_sb.append(t)
            t = cpool.tile([128, NT[st]], f32, tag=f"dlc{st}")
            nc.sync.dma_start(t[:], dlc_d[st])
            dlc_sb.append(t)
        dvd_sb = cpool.tile([128, 2 * NW], f32)
        nc.sync.dma_start(dvd_sb[:], dvd_d)

        iota_f32 = cpool.tile([128, WIN], f32)
        nc.gpsimd.iota(iota_f32[:], pattern=[[1, WIN]], base=0,
                       channel_multiplier=0,
                       allow_small_or_imprecise_dtypes=True)

        if not fast_path:
            ones_sb = cpool.tile([1, 128], f32)
            nc.vector.memset(ones_sb[:], 1.0)
            arow_sb = cpool.tile([1, H], f32)
            nc.sync.dma_start(arow_sb[:], arow_d)
            brow_sb = cpool.tile([1, H], f32)
            nc.sync.dma_start(brow_sb[:], brow_d)
            arep_ps = ps_out.tile([128, H], f32, tag="brd")
            nc.tensor.matmul(arep_ps[:], lhsT=ones_sb[:], rhs=arow_sb[:],
                             start=True, stop=True)
            arep_sb = cpool.tile([128, H], f32)
            nc.scalar.copy(arep_sb[:], arep_ps[:])
            brep_ps = ps_out.tile([128, H], f32, tag="brd")
            nc.tensor.matmul(brep_ps[:], lhsT=ones_sb[:], rhs=brow_sb[:],
                             start=True, stop=True)
            brep_sb = cpool.tile([128, H], f32)
            nc.scalar.copy(brep_sb[:], brep_ps[:])

        # ---- main loop ----
        gx_tiles = [[None] * n_chunks[0], [None] * n_chunks[1]]

        def chunk_tile(st, c):
            if gx_tiles[st][c] is None:
                num = min(CH, NT[st] * TILE - c * CH)
                nblk = num // TILE
                gx = gxpool.tile([128, CH // TILE, TILE], f32r, tag="gx")
                nc.gpsimd.dma_gather(
                    out_ap=gx[:, 0:nblk, :],
                    in_ap=x_in[st],
                    idxs_ap=idx_sb[st][:, c * (CH // 16): c * (CH // 16) + num // 16],
                    num_idxs=num,
                    num_idxs_reg=num,
                    elem_size=F,
                )
                gx_tiles[st][c] = gx
            return gx_tiles[st][c]

        for w in range(NW):
            pagg = ps_agg.tile([128, WIN], f32, tag="pagg")
            n_mm = int(T[0, w] + T[1, w])
            mm_i = 0
            for st in range(2):
                for t in range(int(T[st, w])):
                    gt = int(tile_start[st, w]) + t
                    c, blk = divmod(gt, CH // TILE)
                    gx = chunk_tile(st, c)
                    oh = ohpool.tile([128, WIN], f32r, tag="oh")
                    nc.vector.tensor_scalar(
                        oh[:], iota_f32[:],
                        dlc_sb[st][:, gt:gt + 1], dsc_sb[st][:, gt:gt + 1],
                        op0=Alu.is_equal, op1=Alu.mult,
                    )
                    nc.tensor.matmul(
                        pagg[:],
                        lhsT=gx[:, blk:blk + 1, :],
                        rhs=oh[:],
                        start=(mm_i == 0), stop=(mm_i == n_mm - 1),
                    )
                    mm_i += 1
            agg_sb = aggpool.tile([128, WIN], f32r, tag="aggs")
            nc.scalar.copy(agg_sb[:], pagg[:])

            for h2 in range(2):
                hw = 2 * w + h2
                lhs = agg_sb[:, h2 * 128:(h2 + 1) * 128]
                dv_col = dvd_sb[:, hw:hw + 1]
                ps0 = ps_out.tile([128, H], f32, tag="ps0")
                nc.tensor.matmul(ps0[:], lhsT=lhs, rhs=wt_sb[:],
                                 start=True, stop=True)
                if fast_path:
                    # out = relu(dinv*v) + min(dinv*alpha*v, 0)  (alpha>0, b=0)
                    ps1 = ps_out.tile([128, H], f32, tag="ps1")
                    nc.tensor.matmul(ps1[:], lhsT=lhs, rhs=w1t_sb[:],
                                     start=True, stop=True)
                    pos = eppool.tile([128, H], f32, tag="pos")
                    nc.scalar.activation(pos[:], ps0[:], Act.Relu, scale=dv_col)
                    nega = eppool.tile([128, H], f32, tag="nega")
                    nc.vector.tensor_scalar(nega[:], ps1[:], 0.0, dv_col,
                                            op0=Alu.min, op1=Alu.mult)
                    outt = eppool.tile([128, H], f32, tag="outt")
                    nc.gpsimd.tensor_tensor(outt[:], pos[:], nega[:],
                                            op=Alu.add)
                else:
                    # general: v = dinv*ps0 + b; out = relu(v) + alpha*min(v,0)
                    vb = eppool.tile([128, H], f32, tag="vb")
                    nc.vector.tensor_scalar(vb[:], ps0[:], dv_col, None,
                                            op0=Alu.mult)
                    vb2 = eppool.tile([128, H], f32, tag="vb2")
                    nc.vector.tensor_tensor(vb2[:], vb[:], brep_sb[:],
                                            op=Alu.add)
                    pos = eppool.tile([128, H], f32, tag="pos")
                    nc.scalar.activation(pos[:], vb2[:], Act.Relu)
                    neg = eppool.tile([128, H], f32, tag="neg")
                    nc.vector.tensor_scalar(neg[:], vb2[:], 0.0, None,
                                            op0=Alu.min)
                    nega = eppool.tile([128, H], f32, tag="nega")
                    nc.vector.tensor_tensor(nega[:], neg[:], arep_sb[:],
                                            op=Alu.mult)
                    outt = eppool.tile([128, H], f32, tag="outt")
                    nc.gpsimd.tensor_tensor(outt[:], pos[:], nega[:],
                                            op=Alu.add)
                nc.sync.dma_start(out_d[hw * 128:(hw + 1) * 128, :], outt[:])

    nc.compile()
    return nc


def kernel(x, edge_index, W, b, alpha):
    from concourse.bass_utils import run_bass_kernel_spmd

    t0 = time.time()
    x = np.ascontiguousarray(np.asarray(x, dtype=np.float32))
    W = np.asarray(W, dtype=np.float32)
    b = np.asarray(b, dtype=np.float32)
    alpha = np.asarray(alpha, dtype=np.float32)

    per_core, T, tile_start, NT, dinv = _plan(edge_index)
    fast_path = bool(np.all(b == 0.0) and np.all(alpha > 0.0))

    wt = np.ascontiguousarray(W.T)                       # [F, H]
    w1t = np.ascontiguousarray((alpha[:, None] * W).T)   # [F, H]

    t1 = time.time()
    nc = _build_program(T, tile_start, NT, fast_path)
    t2 = time.time()

    in_maps = []
    for k in range(NC_CORES):
        d = per_core[k]
        m = {
            "x": x, "w_t": wt, "w1_t": w1t,
            "idx0": d[("idx", 0)], "idx1": d[("idx", 1)],
            "dsc0": d[("dsc", 0)], "dsc1": d[("dsc", 1)],
            "dlc0": d[("dlc", 0)], "dlc1": d[("dlc", 1)],
            "dinv_dst": d["dinv_dst"],
        }
        if not fast_path:
            m["alpha_row"] = alpha.reshape(1, H).astype(np.float32)
            m["b_row"] = b.reshape(1, H).astype(np.float32)
        in_maps.append(m)

    trace = bool(int(os.environ.get("GCN_BASS_TRACE", "0")))
    res = run_bass_kernel_spmd(nc, in_maps, core_ids=list(range(NC_CORES)),
                               trace=trace)
    t3 = time.time()

    outs = np.stack([res.results[k]["out"][:ND] for k in range(NC_CORES)])  # [8, 6250, H]
    out_full = outs.transpose(1, 0, 2).reshape(N, H)
    t4 = time.time()

    last_run_info.update({
        "exec_time_ns": res.exec_time_ns,
        "plan_s": t1 - t0, "build_s": t2 - t1, "run_s": t3 - t2,
        "unshard_s": t4 - t3, "fast_path": fast_path,
        "NT": NT, "trace": trace,
    })
    return out_full

